# revision 14
# baseline (speedup 1.0000x reference)
"""Trainium2 Bass kernel for the MLPConstructor2 adjacency problem.

Computes, per batch b (one NeuronCore each, 8-way data parallel over B):
    adj[i, j] = tanh(relu(x1_i @ w1 + x2_j @ w2 + b))
for the four (spatial/temporal) quadrants of a (2560, 2560) output,
stored as bf16 (widened to f32 on the host; quantization error ~8e-3
against a 2e-2 gate, dominated by bf16 rounding of the col stats).

The output is an outer broadcast-sum of per-row and per-column scalar
vectors. The kernel is ScalarE-bound: every output element takes one
tanh slot (1 elem/cycle/lane at 1.2 GHz, any dtype -> ~43 us floor),
with the 13.1 MB/core bf16 store (~40 us at ~330 GB/s) just under it.

Design:
- x is staged twice, in (t p) layout for the row-side stats (so each
  128-row output tile's biases land on partitions directly) and in (p t)
  layout for the col-side stats (so the stat tile streams out to a DRAM
  scratch contiguously in row order -- no transpose anywhere).
- Dot-product stats are mul/reduce on VectorE; quadrant biases are
  folded into the row-side stats (off the col critical path).
- Column vectors are replicated across partitions by bouncing each stat
  through a per-quadrant DRAM scratch and reloading with a
  partition-step-0 broadcast AP, in bf16 to halve the transfer.
- Every SBUF tile is written by exactly ONE producer DMA/op so
  dependency tracking never serializes on false tile-level conflicts.
- Ring discipline (HWDGE dma_start costs ~0.65us issue + 0.65us DGE +
  0.9us sem-propagation; SWDGE broadcasts cost ~3.2us): the Sync ring
  carries the critical chain (x stages, scratch stores, col broadcasts)
  then even-tile stores; the Scalar ring carries the early weight/bias
  broadcasts (done before its first TANH at ~15us); the GpSimd ring
  takes the late-needed weights and odd-tile stores. A dummy [128,1]
  tanh at t=0 preloads the activation table.
- Main loop per 128-row output tile: 2 ScalarE tanh activations
  (per-quadrant per-partition row bias), 1 VectorE bf16 relu in place,
  one 655 KB contiguous bf16 store; the final store is split 3 ways
  across rings to shrink the drain tail.
"""

import numpy as np
from contextlib import ExitStack

import concourse.bacc as bacc
import concourse.mybir as mybir
import concourse.tile as tile
from concourse.bass_utils import run_bass_kernel_spmd

B, N, T, D = 8, 2048, 512, 32
W = N + T            # 2560
NT, TT = N // 128, T // 128   # 16, 4 row-tiles
F32 = mybir.dt.float32
BF16 = mybir.dt.bfloat16
QUADS = ("ss", "st", "ts", "tt")


def _emit(tc, sp, tm, ws, scr, adj):
    nc = tc.nc
    AF = mybir.ActivationFunctionType
    OP = mybir.AluOpType
    with ExitStack() as ctx:
        const = ctx.enter_context(tc.tile_pool(name="const", bufs=1))
        outp = ctx.enter_context(tc.tile_pool(name="outp", bufs=8))

        # dummy tanh: pulls ACT_TABLE_LOAD off the first real TANH
        dummy = const.tile([128, 1], F32, name="dummy")
        nc.vector.memset(dummy[:], 0.0)
        nc.scalar.activation(dummy[:], dummy[:], AF.Tanh)

        # ---- stage inputs on the Sync ring, fast (p t) layouts first ------
        # (p t): row p*nt+t at [p, t*D:(t+1)*D]; (t p): row t*128+p there.
        x_sp_pt = const.tile([128, NT * D], F32, name="x_sp_pt")
        nc.sync.dma_start(x_sp_pt[:], sp.rearrange("(p t) d -> p t d", p=128))
        x_tm_pt = const.tile([128, TT * D], F32, name="x_tm_pt")
        nc.sync.dma_start(x_tm_pt[:], tm.rearrange("(p t) d -> p t d", p=128))
        x_sp_tp = const.tile([128, NT * D], F32, name="x_sp_tp")
        nc.sync.dma_start(x_sp_tp[:], sp.rearrange("(t p) d -> p t d", p=128))
        x_tm_tp = const.tile([128, TT * D], F32, name="x_tm_tp")
        nc.sync.dma_start(x_tm_tp[:], tm.rearrange("(t p) d -> p t d", p=128))

        # ---- weight/bias broadcasts, one tile per DMA ---------------------
        def wtile(name, nm, half, eng):
            t = const.tile([128, D], F32, name=name)
            src = ws[f"w_{nm}"][half * D : (half + 1) * D]
            eng.dma_start(t[:], src.unsqueeze(0).broadcast_to((128, D)))
            return t

        def btile(name, nm, eng):
            t = const.tile([128, 1], F32, name=name)
            eng.dma_start(t[:], ws[f"b_{nm}"].unsqueeze(0).broadcast_to((128, 1)))
            return t

        # Scalar ring: everything the first ~6 TANHs depend on
        w_css = wtile("w_css", "ss", 1, nc.scalar)
        b_ss = btile("b_ss", "ss", nc.scalar)
        w_rss = wtile("w_rss", "ss", 0, nc.scalar)
        b_st = btile("b_st", "st", nc.scalar)
        w_rst = wtile("w_rst", "st", 0, nc.scalar)
        w_cts = wtile("w_cts", "ts", 1, nc.scalar)
        b_ts = btile("b_ts", "ts", nc.scalar)
        b_tt = btile("b_tt", "tt", nc.scalar)
        # GpSimd ring (slow SWDGE broadcasts, needed later)
        w_cst = wtile("w_cst", "st", 1, nc.gpsimd)
        w_ctt = wtile("w_ctt", "tt", 1, nc.gpsimd)
        w_rts = wtile("w_rts", "ts", 0, nc.gpsimd)
        w_rtt = wtile("w_rtt", "tt", 0, nc.gpsimd)

        # ---- stats on VectorE: mul + reduce over D ------------------------
        col_sp_n = const.tile([128, N], BF16, name="col_sp_n")
        col_sp_t = const.tile([128, T], BF16, name="col_sp_t")
        col_tm_n = const.tile([128, N], BF16, name="col_tm_n")
        col_tm_t = const.tile([128, T], BF16, name="col_tm_t")

        def cstat(x, nt, w, scr_t, col_dst, name):
            # col-side slot in (p t) layout: mul/reduce (bf16 out; total
            # quantization measured 8e-3 vs the 2e-2 gate), bounce through
            # DRAM scratch, partition-broadcast reload into col_dst.
            prod = const.tile([128, nt * D], F32, name=f"cprod_{name}")
            x3 = x[:].rearrange("p (t d) -> p t d", t=nt)
            p3 = prod[:].rearrange("p (t d) -> p t d", t=nt)
            nc.vector.tensor_tensor(
                p3, x3, w[:].unsqueeze(1).broadcast_to((128, nt, D)), OP.mult
            )
            st = const.tile([128, nt], BF16, name=f"cstat_{name}")
            with nc.allow_low_precision(reason="bf16 col stats; 8e-3 err vs 2e-2 gate"):
                nc.vector.tensor_reduce(st[:], p3, axis=mybir.AxisListType.X, op=OP.add)
            n = 128 * nt
            nc.sync.dma_start(scr_t[0:n], st[:])
            nc.sync.dma_start(col_dst, scr_t[0:n].unsqueeze(0).broadcast_to((128, n)))

        def rstat(x, nt, w, b, name):
            # row-side slot in (t p) layout, quadrant bias folded in
            prod = const.tile([128, nt * D], F32, name=f"rprod_{name}")
            x3 = x[:].rearrange("p (t d) -> p t d", t=nt)
            p3 = prod[:].rearrange("p (t d) -> p t d", t=nt)
            nc.vector.tensor_tensor(
                p3, x3, w[:].unsqueeze(1).broadcast_to((128, nt, D)), OP.mult
            )
            r = const.tile([128, nt], F32, name=f"r_{name}")
            nc.vector.tensor_reduce(r[:], p3, axis=mybir.AxisListType.X, op=OP.add)
            nc.vector.tensor_scalar_add(r[:], r[:], b[:])
            return r

        # critical order: col_sp halves and r_ss/r_st feed the first TANHs
        cstat(x_sp_pt, NT, w_css, scr["ss"], col_sp_n[:], "ss")
        cstat(x_tm_pt, TT, w_cst, scr["st"], col_sp_t[:], "st")
        r_ss = rstat(x_sp_tp, NT, w_rss, b_ss, "ss")
        r_st = rstat(x_sp_tp, NT, w_rst, b_st, "st")
        cstat(x_sp_pt, NT, w_cts, scr["ts"], col_tm_n[:], "ts")
        cstat(x_tm_pt, TT, w_ctt, scr["tt"], col_tm_t[:], "tt")
        r_ts = rstat(x_tm_tp, TT, w_rts, b_ts, "ts")
        r_tt = rstat(x_tm_tp, TT, w_rtt, b_tt, "tt")

        # ---- main loop: 20 output row-tiles of [128, 2560] ----------------
        def row_block(k, row0, col_n, col_t, r_n, r_t, t, last=False):
            ot = outp.tile([128, W], BF16, name=f"ot{k}", tag="ot")
            nc.scalar.activation(
                ot[:, 0:N], col_n[:], AF.Tanh, bias=r_n[:, t : t + 1]
            )
            nc.scalar.activation(
                ot[:, N:W], col_t[:], AF.Tanh, bias=r_t[:, t : t + 1]
            )
            nc.vector.tensor_scalar_max(ot[:], ot[:], 0.0)
            if last:
                # split the final store across all rings to shrink the tail
                for eng, lo, hi in ((nc.sync, 0, 1024), (nc.gpsimd, 1024, 2048),
                                    (nc.scalar, 2048, W)):
                    eng.dma_start(adj[row0 : row0 + 128, lo:hi], ot[:, lo:hi])
            else:
                eng = nc.sync if k % 2 == 0 else nc.gpsimd
                eng.dma_start(adj[row0 : row0 + 128, :], ot[:])

        for t in range(NT):
            row_block(t, 128 * t, col_sp_n, col_sp_t, r_ss, r_st, t)
        for t in range(TT):
            row_block(NT + t, N + 128 * t, col_tm_n, col_tm_t, r_ts, r_tt, t,
                      last=(t == TT - 1))


def build_nc(num_devices=8):
    nc = bacc.Bacc(
        "TRN2",
        target_bir_lowering=False,
        debug=False,
        enable_asserts=False,
        num_devices=num_devices,
    )
    sp = nc.dram_tensor("spatial_nodes", (N, D), F32, kind="ExternalInput").ap()
    tm = nc.dram_tensor("temporal_nodes", (T, D), F32, kind="ExternalInput").ap()
    ws = {}
    for nm in QUADS:
        ws[f"w_{nm}"] = nc.dram_tensor(f"w_{nm}", (2 * D,), F32, kind="ExternalInput").ap()
        ws[f"b_{nm}"] = nc.dram_tensor(f"b_{nm}", (1,), F32, kind="ExternalInput").ap()
    scr = {
        nm: nc.dram_tensor(f"scr_{nm}", (sz,), BF16, kind="Internal").ap()
        for nm, sz in (("ss", N), ("st", T), ("ts", N), ("tt", T))
    }
    adj = nc.dram_tensor("adj", (W, W), BF16, kind="ExternalOutput").ap()

    with tile.TileContext(nc) as tc:
        _emit(tc, sp, tm, ws, scr, adj)
    nc.compile()
    return nc


def make_in_maps(inputs):
    in_maps = []
    for b in range(B):
        m = {
            "spatial_nodes": np.ascontiguousarray(inputs["spatial_nodes"][b], np.float32),
            "temporal_nodes": np.ascontiguousarray(inputs["temporal_nodes"][b], np.float32),
        }
        for nm in QUADS:
            m[f"w_{nm}"] = np.ascontiguousarray(inputs[f"w_{nm}"], np.float32)
            m[f"b_{nm}"] = np.ascontiguousarray(inputs[f"b_{nm}"], np.float32)
        in_maps.append(m)
    return in_maps


_NC = {}


def run(inputs, trace=False, trace_cores=None):
    if 8 not in _NC:
        _NC[8] = build_nc(8)
    res = run_bass_kernel_spmd(
        _NC[8], make_in_maps(inputs), core_ids=list(range(B)), trace=trace,
        trace_cores=trace_cores,
    )
    out = np.stack(
        [np.asarray(res.results[i]["adj"]).astype(np.float32) for i in range(B)],
        axis=0,
    )
    return out, res


def kernel(**inputs) -> np.ndarray:
    out, _ = run(inputs, trace=False)
    return out


# revision 15
# speedup vs baseline: 1.0884x; 1.0884x over previous
"""Trainium2 Bass kernel for the MLPConstructor2 adjacency problem.

Computes, per batch b (one NeuronCore each, 8-way data parallel over B):
    adj[i, j] = tanh(relu(x1_i @ w1 + x2_j @ w2 + b))
for the four (spatial/temporal) quadrants of a (2560, 2560) output,
stored as bf16 (widened to f32 on the host; quantization error ~8e-3
against a 2e-2 gate, dominated by bf16 rounding of the col stats).

The output is an outer broadcast-sum of per-row and per-column scalar
vectors. The kernel is ScalarE-bound: every output element takes one
tanh slot (1 elem/cycle/lane at 1.2 GHz, any dtype -> ~43 us floor),
with the 13.1 MB/core bf16 store (~40 us at ~330 GB/s) just under it.

Design:
- x is staged twice, in (t p) layout for the row-side stats (so each
  128-row output tile's biases land on partitions directly) and in (p t)
  layout for the col-side stats (so the stat tile streams out to a DRAM
  scratch contiguously in row order -- no transpose anywhere).
- Dot-product stats are mul/reduce on VectorE; quadrant biases are
  folded into the row-side stats (off the col critical path).
- Column vectors are replicated across partitions by bouncing each stat
  through a per-quadrant DRAM scratch and reloading with a
  partition-step-0 broadcast AP, in bf16 to halve the transfer.
- Every SBUF tile is written by exactly ONE producer DMA/op so
  dependency tracking never serializes on false tile-level conflicts.
- Ring discipline (HWDGE dma_start costs ~0.65us issue + 0.65us DGE +
  0.9us sem-propagation; SWDGE broadcasts cost ~3.2us): the Sync ring
  carries the critical chain (x stages, scratch stores, col broadcasts)
  then even-tile stores; the Scalar ring carries the early weight/bias
  broadcasts (done before its first TANH at ~15us); the GpSimd ring
  takes the late-needed weights and odd-tile stores. A dummy [128,1]
  tanh at t=0 preloads the activation table.
- Main loop per 128-row output tile: 2 ScalarE tanh activations
  (per-quadrant per-partition row bias), 1 VectorE bf16 relu in place,
  one 655 KB contiguous bf16 store; the final store is split 3 ways
  across rings to shrink the drain tail.
"""

import numpy as np
from contextlib import ExitStack

import concourse.bacc as bacc
import concourse.mybir as mybir
import concourse.tile as tile
from concourse.bass_utils import run_bass_kernel_spmd

B, N, T, D = 8, 2048, 512, 32
W = N + T            # 2560
NT, TT = N // 128, T // 128   # 16, 4 row-tiles
F32 = mybir.dt.float32
BF16 = mybir.dt.bfloat16
QUADS = ("ss", "st", "ts", "tt")


def _emit(tc, sp, tm, ws, scr, adj):
    nc = tc.nc
    AF = mybir.ActivationFunctionType
    OP = mybir.AluOpType
    with ExitStack() as ctx:
        const = ctx.enter_context(tc.tile_pool(name="const", bufs=1))
        outp = ctx.enter_context(tc.tile_pool(name="outp", bufs=8))

        # dummy tanh: pulls ACT_TABLE_LOAD off the first real TANH
        dummy = const.tile([128, 1], F32, name="dummy")
        nc.vector.memset(dummy[:], 0.0)
        nc.scalar.activation(dummy[:], dummy[:], AF.Tanh)

        # ---- stage inputs on the Sync ring, fast (p t) layouts first ------
        # (p t): row p*nt+t at [p, t*D:(t+1)*D]; (t p): row t*128+p there.
        x_sp_pt = const.tile([128, NT * D], F32, name="x_sp_pt")
        nc.sync.dma_start(x_sp_pt[:], sp.rearrange("(p t) d -> p t d", p=128))
        x_tm_pt = const.tile([128, TT * D], F32, name="x_tm_pt")
        nc.sync.dma_start(x_tm_pt[:], tm.rearrange("(p t) d -> p t d", p=128))
        x_sp_tp = const.tile([128, NT * D], F32, name="x_sp_tp")
        nc.sync.dma_start(x_sp_tp[:], sp.rearrange("(t p) d -> p t d", p=128))
        x_tm_tp = const.tile([128, TT * D], F32, name="x_tm_tp")
        nc.sync.dma_start(x_tm_tp[:], tm.rearrange("(t p) d -> p t d", p=128))

        # ---- weight/bias broadcasts, one tile per DMA ---------------------
        def wtile(name, nm, half, eng):
            t = const.tile([128, D], F32, name=name)
            src = ws[f"w_{nm}"][half * D : (half + 1) * D]
            eng.dma_start(t[:], src.unsqueeze(0).broadcast_to((128, D)))
            return t

        def btile(name, nm, eng):
            t = const.tile([128, 1], F32, name=name)
            eng.dma_start(t[:], ws[f"b_{nm}"].unsqueeze(0).broadcast_to((128, 1)))
            return t

        # Scalar ring: everything the first ~6 TANHs depend on
        w_css = wtile("w_css", "ss", 1, nc.scalar)
        b_ss = btile("b_ss", "ss", nc.scalar)
        w_rss = wtile("w_rss", "ss", 0, nc.scalar)
        b_st = btile("b_st", "st", nc.scalar)
        w_rst = wtile("w_rst", "st", 0, nc.scalar)
        w_cts = wtile("w_cts", "ts", 1, nc.scalar)
        b_ts = btile("b_ts", "ts", nc.scalar)
        b_tt = btile("b_tt", "tt", nc.scalar)
        # GpSimd ring (slow SWDGE broadcasts, needed later)
        w_cst = wtile("w_cst", "st", 1, nc.gpsimd)
        w_ctt = wtile("w_ctt", "tt", 1, nc.gpsimd)
        w_rts = wtile("w_rts", "ts", 0, nc.gpsimd)
        w_rtt = wtile("w_rtt", "tt", 0, nc.gpsimd)

        # ---- stats on VectorE: mul + reduce over D ------------------------
        col_sp_n = const.tile([128, N], BF16, name="col_sp_n")
        col_sp_t = const.tile([128, T], BF16, name="col_sp_t")
        col_tm_n = const.tile([128, N], BF16, name="col_tm_n")
        col_tm_t = const.tile([128, T], BF16, name="col_tm_t")

        def cstat(x, nt, w, scr_t, col_dst, name):
            # col-side slot in (p t) layout: mul/reduce (bf16 out; total
            # quantization measured 8e-3 vs the 2e-2 gate), bounce through
            # DRAM scratch, partition-broadcast reload into col_dst.
            prod = const.tile([128, nt * D], F32, name=f"cprod_{name}")
            x3 = x[:].rearrange("p (t d) -> p t d", t=nt)
            p3 = prod[:].rearrange("p (t d) -> p t d", t=nt)
            nc.vector.tensor_tensor(
                p3, x3, w[:].unsqueeze(1).broadcast_to((128, nt, D)), OP.mult
            )
            st = const.tile([128, nt], BF16, name=f"cstat_{name}")
            with nc.allow_low_precision(reason="bf16 col stats; 8e-3 err vs 2e-2 gate"):
                nc.vector.tensor_reduce(st[:], p3, axis=mybir.AxisListType.X, op=OP.add)
            n = 128 * nt
            nc.sync.dma_start(scr_t[0:n], st[:])
            nc.sync.dma_start(col_dst, scr_t[0:n].unsqueeze(0).broadcast_to((128, n)))

        def rstat(x, nt, w, b, name):
            # row-side slot in (t p) layout, quadrant bias folded in
            prod = const.tile([128, nt * D], F32, name=f"rprod_{name}")
            x3 = x[:].rearrange("p (t d) -> p t d", t=nt)
            p3 = prod[:].rearrange("p (t d) -> p t d", t=nt)
            nc.vector.tensor_tensor(
                p3, x3, w[:].unsqueeze(1).broadcast_to((128, nt, D)), OP.mult
            )
            r = const.tile([128, nt], F32, name=f"r_{name}")
            nc.vector.tensor_reduce(r[:], p3, axis=mybir.AxisListType.X, op=OP.add)
            nc.vector.tensor_scalar_add(r[:], r[:], b[:])
            return r

        # critical order: col_sp halves and r_ss/r_st feed the first TANHs.
        # rstats are emitted early so their bias adds get early slots in the
        # Vector static order (first-TANH sem waits are stream-count based).
        cstat(x_sp_pt, NT, w_css, scr["ss"], col_sp_n[:], "ss")
        r_ss = rstat(x_sp_tp, NT, w_rss, b_ss, "ss")
        r_st = rstat(x_sp_tp, NT, w_rst, b_st, "st")
        cstat(x_tm_pt, TT, w_cst, scr["st"], col_sp_t[:], "st")
        cstat(x_sp_pt, NT, w_cts, scr["ts"], col_tm_n[:], "ts")
        cstat(x_tm_pt, TT, w_ctt, scr["tt"], col_tm_t[:], "tt")
        r_ts = rstat(x_tm_tp, TT, w_rts, b_ts, "ts")
        r_tt = rstat(x_tm_tp, TT, w_rtt, b_tt, "tt")

        # ---- main loop: 20 output row-tiles of [128, 2560] ----------------
        def row_block(k, row0, col_n, col_t, r_n, r_t, t, last=False):
            ot = outp.tile([128, W], BF16, name=f"ot{k}", tag="ot")
            nc.scalar.activation(
                ot[:, 0:N], col_n[:], AF.Tanh, bias=r_n[:, t : t + 1]
            )
            nc.scalar.activation(
                ot[:, N:W], col_t[:], AF.Tanh, bias=r_t[:, t : t + 1]
            )
            nc.vector.tensor_scalar_max(ot[:], ot[:], 0.0)
            if last:
                # split the final store across all rings to shrink the tail
                for eng, lo, hi in ((nc.sync, 0, 1024), (nc.gpsimd, 1024, 2048),
                                    (nc.scalar, 2048, W)):
                    eng.dma_start(adj[row0 : row0 + 128, lo:hi], ot[:, lo:hi])
            else:
                eng = nc.sync if k % 2 == 0 else nc.gpsimd
                eng.dma_start(adj[row0 : row0 + 128, :], ot[:])

        for t in range(NT):
            row_block(t, 128 * t, col_sp_n, col_sp_t, r_ss, r_st, t)
        for t in range(TT):
            row_block(NT + t, N + 128 * t, col_tm_n, col_tm_t, r_ts, r_tt, t,
                      last=(t == TT - 1))


def build_nc(num_devices=8):
    nc = bacc.Bacc(
        "TRN2",
        target_bir_lowering=False,
        debug=False,
        enable_asserts=False,
        num_devices=num_devices,
    )
    sp = nc.dram_tensor("spatial_nodes", (N, D), F32, kind="ExternalInput").ap()
    tm = nc.dram_tensor("temporal_nodes", (T, D), F32, kind="ExternalInput").ap()
    ws = {}
    for nm in QUADS:
        ws[f"w_{nm}"] = nc.dram_tensor(f"w_{nm}", (2 * D,), F32, kind="ExternalInput").ap()
        ws[f"b_{nm}"] = nc.dram_tensor(f"b_{nm}", (1,), F32, kind="ExternalInput").ap()
    scr = {
        nm: nc.dram_tensor(f"scr_{nm}", (sz,), BF16, kind="Internal").ap()
        for nm, sz in (("ss", N), ("st", T), ("ts", N), ("tt", T))
    }
    adj = nc.dram_tensor("adj", (W, W), BF16, kind="ExternalOutput").ap()

    with tile.TileContext(nc) as tc:
        _emit(tc, sp, tm, ws, scr, adj)
    nc.compile()
    return nc


def make_in_maps(inputs):
    in_maps = []
    for b in range(B):
        m = {
            "spatial_nodes": np.ascontiguousarray(inputs["spatial_nodes"][b], np.float32),
            "temporal_nodes": np.ascontiguousarray(inputs["temporal_nodes"][b], np.float32),
        }
        for nm in QUADS:
            m[f"w_{nm}"] = np.ascontiguousarray(inputs[f"w_{nm}"], np.float32)
            m[f"b_{nm}"] = np.ascontiguousarray(inputs[f"b_{nm}"], np.float32)
        in_maps.append(m)
    return in_maps


_NC = {}


def run(inputs, trace=False, trace_cores=None):
    if 8 not in _NC:
        _NC[8] = build_nc(8)
    res = run_bass_kernel_spmd(
        _NC[8], make_in_maps(inputs), core_ids=list(range(B)), trace=trace,
        trace_cores=trace_cores,
    )
    out = np.stack(
        [np.asarray(res.results[i]["adj"]).astype(np.float32) for i in range(B)],
        axis=0,
    )
    return out, res


def kernel(**inputs) -> np.ndarray:
    out, _ = run(inputs, trace=False)
    return out


# revision 16
# speedup vs baseline: 1.1223x; 1.0312x over previous
"""Trainium2 Bass kernel for the MLPConstructor2 adjacency problem.

Computes, per batch b (one NeuronCore each, 8-way data parallel over B):
    adj[i, j] = tanh(relu(x1_i @ w1 + x2_j @ w2 + b))
for the four (spatial/temporal) quadrants of a (2560, 2560) output,
stored as bf16 (widened to f32 on the host; quantization error ~8e-3
against a 2e-2 gate, dominated by bf16 rounding of the col stats).

The output is an outer broadcast-sum of per-row and per-column scalar
vectors. The kernel is ScalarE-bound: every output element takes one
tanh slot (1 elem/cycle/lane at 1.2 GHz, any dtype -> ~43 us floor),
with the 13.1 MB/core bf16 store (~40 us at ~330 GB/s) just under it.

Design:
- x is staged twice, in (t p) layout for the row-side stats (so each
  128-row output tile's biases land on partitions directly) and in (p t)
  layout for the col-side stats (so the stat tile streams out to a DRAM
  scratch contiguously in row order -- no transpose anywhere).
- Dot-product stats are mul/reduce on VectorE; quadrant biases are
  folded into the row-side stats (off the col critical path).
- Column vectors are replicated across partitions by bouncing each stat
  through a per-quadrant DRAM scratch and reloading with a
  partition-step-0 broadcast AP, in bf16 to halve the transfer.
- Every SBUF tile is written by exactly ONE producer DMA/op so
  dependency tracking never serializes on false tile-level conflicts.
- Ring discipline (HWDGE dma_start costs ~0.65us issue + 0.65us DGE +
  0.9us sem-propagation; SWDGE broadcasts cost ~3.2us): the Sync ring
  carries the critical chain (x stages, scratch stores, col broadcasts)
  then even-tile stores; the Scalar ring carries the early weight/bias
  broadcasts (done before its first TANH at ~15us); the GpSimd ring
  takes the late-needed weights and odd-tile stores. A dummy [128,1]
  tanh at t=0 preloads the activation table.
- Main loop per 128-row output tile: 2 ScalarE tanh activations
  (per-quadrant per-partition row bias), 1 VectorE bf16 relu in place,
  one 655 KB contiguous bf16 store; the final store is split 3 ways
  across rings to shrink the drain tail.
"""

import numpy as np
from contextlib import ExitStack

import concourse.bacc as bacc
import concourse.mybir as mybir
import concourse.tile as tile
from concourse.bass_utils import run_bass_kernel_spmd

B, N, T, D = 8, 2048, 512, 32
W = N + T            # 2560
NT, TT = N // 128, T // 128   # 16, 4 row-tiles
F32 = mybir.dt.float32
BF16 = mybir.dt.bfloat16
QUADS = ("ss", "st", "ts", "tt")


def _emit(tc, sp, tm, ws, scr, adj):
    nc = tc.nc
    AF = mybir.ActivationFunctionType
    OP = mybir.AluOpType
    with ExitStack() as ctx:
        const = ctx.enter_context(tc.tile_pool(name="const", bufs=1))
        outp = ctx.enter_context(tc.tile_pool(name="outp", bufs=8))

        # dummy tanh: pulls ACT_TABLE_LOAD off the first real TANH
        dummy = const.tile([128, 1], F32, name="dummy")
        nc.vector.memset(dummy[:], 0.0)
        nc.scalar.activation(dummy[:], dummy[:], AF.Tanh)

        # ---- stage inputs on the Sync ring, fast (p t) layouts first ------
        # (p t): row p*nt+t at [p, t*D:(t+1)*D]; (t p): row t*128+p there.
        x_sp_pt = const.tile([128, NT * D], F32, name="x_sp_pt")
        nc.sync.dma_start(x_sp_pt[:], sp.rearrange("(p t) d -> p t d", p=128))
        x_tm_pt = const.tile([128, TT * D], F32, name="x_tm_pt")
        nc.sync.dma_start(x_tm_pt[:], tm.rearrange("(p t) d -> p t d", p=128))
        # the (t p) stages are 128B-chunk scattered reads (~7us for x_sp);
        # issue x_sp's in 4 pieces so the in-order Sync sequencer can slip
        # the critical scratch-store/broadcast DMAs in between.
        x_sp_tp = const.tile([128, NT * D], F32, name="x_sp_tp")
        sp_tp = sp.rearrange("(t p) d -> p t d", p=128)
        for c in range(4):
            q = NT // 4
            nc.sync.dma_start(
                x_sp_tp[:].rearrange("p (t d) -> p t d", t=NT)[:, c * q : (c + 1) * q],
                sp_tp[:, c * q : (c + 1) * q],
            )
        x_tm_tp = const.tile([128, TT * D], F32, name="x_tm_tp")
        nc.sync.dma_start(x_tm_tp[:], tm.rearrange("(t p) d -> p t d", p=128))

        # ---- weight/bias broadcasts, one tile per DMA ---------------------
        def wtile(name, nm, half, eng):
            t = const.tile([128, D], F32, name=name)
            src = ws[f"w_{nm}"][half * D : (half + 1) * D]
            eng.dma_start(t[:], src.unsqueeze(0).broadcast_to((128, D)))
            return t

        def btile(name, nm, eng):
            t = const.tile([128, 1], F32, name=name)
            eng.dma_start(t[:], ws[f"b_{nm}"].unsqueeze(0).broadcast_to((128, 1)))
            return t

        # Scalar ring: everything the first ~6 TANHs depend on
        w_css = wtile("w_css", "ss", 1, nc.scalar)
        b_ss = btile("b_ss", "ss", nc.scalar)
        w_rss = wtile("w_rss", "ss", 0, nc.scalar)
        b_st = btile("b_st", "st", nc.scalar)
        w_rst = wtile("w_rst", "st", 0, nc.scalar)
        w_cts = wtile("w_cts", "ts", 1, nc.scalar)
        b_ts = btile("b_ts", "ts", nc.scalar)
        b_tt = btile("b_tt", "tt", nc.scalar)
        # GpSimd ring (slow SWDGE broadcasts, needed later)
        w_cst = wtile("w_cst", "st", 1, nc.gpsimd)
        w_ctt = wtile("w_ctt", "tt", 1, nc.gpsimd)
        w_rts = wtile("w_rts", "ts", 0, nc.gpsimd)
        w_rtt = wtile("w_rtt", "tt", 0, nc.gpsimd)

        # ---- stats on VectorE: mul + reduce over D ------------------------
        col_sp_n = const.tile([128, N], BF16, name="col_sp_n")
        col_sp_t = const.tile([128, T], BF16, name="col_sp_t")
        col_tm_n = const.tile([128, N], BF16, name="col_tm_n")
        col_tm_t = const.tile([128, T], BF16, name="col_tm_t")

        def cstat(x, nt, w, scr_t, col_dst, name):
            # col-side slot in (p t) layout: mul/reduce (bf16 out; total
            # quantization measured 8e-3 vs the 2e-2 gate), bounce through
            # DRAM scratch, partition-broadcast reload into col_dst.
            prod = const.tile([128, nt * D], F32, name=f"cprod_{name}")
            x3 = x[:].rearrange("p (t d) -> p t d", t=nt)
            p3 = prod[:].rearrange("p (t d) -> p t d", t=nt)
            nc.vector.tensor_tensor(
                p3, x3, w[:].unsqueeze(1).broadcast_to((128, nt, D)), OP.mult
            )
            st = const.tile([128, nt], BF16, name=f"cstat_{name}")
            with nc.allow_low_precision(reason="bf16 col stats; 8e-3 err vs 2e-2 gate"):
                nc.vector.tensor_reduce(st[:], p3, axis=mybir.AxisListType.X, op=OP.add)
            n = 128 * nt
            nc.sync.dma_start(scr_t[0:n], st[:])
            nc.sync.dma_start(col_dst, scr_t[0:n].unsqueeze(0).broadcast_to((128, n)))

        def rstat(x, nt, w, b, name):
            # row-side slot in (t p) layout, quadrant bias folded in
            prod = const.tile([128, nt * D], F32, name=f"rprod_{name}")
            x3 = x[:].rearrange("p (t d) -> p t d", t=nt)
            p3 = prod[:].rearrange("p (t d) -> p t d", t=nt)
            nc.vector.tensor_tensor(
                p3, x3, w[:].unsqueeze(1).broadcast_to((128, nt, D)), OP.mult
            )
            r = const.tile([128, nt], F32, name=f"r_{name}")
            nc.vector.tensor_reduce(r[:], p3, axis=mybir.AxisListType.X, op=OP.add)
            nc.vector.tensor_scalar_add(r[:], r[:], b[:])
            return r

        # critical order: col_sp halves and r_ss/r_st feed the first TANHs.
        # rstats are emitted early so their bias adds get early slots in the
        # Vector static order (first-TANH sem waits are stream-count based).
        cstat(x_sp_pt, NT, w_css, scr["ss"], col_sp_n[:], "ss")
        r_ss = rstat(x_sp_tp, NT, w_rss, b_ss, "ss")
        r_st = rstat(x_sp_tp, NT, w_rst, b_st, "st")
        cstat(x_tm_pt, TT, w_cst, scr["st"], col_sp_t[:], "st")
        cstat(x_sp_pt, NT, w_cts, scr["ts"], col_tm_n[:], "ts")
        cstat(x_tm_pt, TT, w_ctt, scr["tt"], col_tm_t[:], "tt")
        r_ts = rstat(x_tm_tp, TT, w_rts, b_ts, "ts")
        r_tt = rstat(x_tm_tp, TT, w_rtt, b_tt, "tt")

        # ---- main loop: 20 output row-tiles of [128, 2560] ----------------
        def row_block(k, row0, col_n, col_t, r_n, r_t, t, last=False):
            ot = outp.tile([128, W], BF16, name=f"ot{k}", tag="ot")
            nc.scalar.activation(
                ot[:, 0:N], col_n[:], AF.Tanh, bias=r_n[:, t : t + 1]
            )
            nc.scalar.activation(
                ot[:, N:W], col_t[:], AF.Tanh, bias=r_t[:, t : t + 1]
            )
            nc.vector.tensor_scalar_max(ot[:], ot[:], 0.0)
            if last:
                # split the final store across all rings to shrink the tail
                for eng, lo, hi in ((nc.sync, 0, 1024), (nc.gpsimd, 1024, 2048),
                                    (nc.scalar, 2048, W)):
                    eng.dma_start(adj[row0 : row0 + 128, lo:hi], ot[:, lo:hi])
            else:
                eng = nc.sync if k % 2 == 0 else nc.gpsimd
                eng.dma_start(adj[row0 : row0 + 128, :], ot[:])

        for t in range(NT):
            row_block(t, 128 * t, col_sp_n, col_sp_t, r_ss, r_st, t)
        for t in range(TT):
            row_block(NT + t, N + 128 * t, col_tm_n, col_tm_t, r_ts, r_tt, t,
                      last=(t == TT - 1))


def build_nc(num_devices=8):
    nc = bacc.Bacc(
        "TRN2",
        target_bir_lowering=False,
        debug=False,
        enable_asserts=False,
        num_devices=num_devices,
    )
    sp = nc.dram_tensor("spatial_nodes", (N, D), F32, kind="ExternalInput").ap()
    tm = nc.dram_tensor("temporal_nodes", (T, D), F32, kind="ExternalInput").ap()
    ws = {}
    for nm in QUADS:
        ws[f"w_{nm}"] = nc.dram_tensor(f"w_{nm}", (2 * D,), F32, kind="ExternalInput").ap()
        ws[f"b_{nm}"] = nc.dram_tensor(f"b_{nm}", (1,), F32, kind="ExternalInput").ap()
    scr = {
        nm: nc.dram_tensor(f"scr_{nm}", (sz,), BF16, kind="Internal").ap()
        for nm, sz in (("ss", N), ("st", T), ("ts", N), ("tt", T))
    }
    adj = nc.dram_tensor("adj", (W, W), BF16, kind="ExternalOutput").ap()

    with tile.TileContext(nc) as tc:
        _emit(tc, sp, tm, ws, scr, adj)
    nc.compile()
    return nc


def make_in_maps(inputs):
    in_maps = []
    for b in range(B):
        m = {
            "spatial_nodes": np.ascontiguousarray(inputs["spatial_nodes"][b], np.float32),
            "temporal_nodes": np.ascontiguousarray(inputs["temporal_nodes"][b], np.float32),
        }
        for nm in QUADS:
            m[f"w_{nm}"] = np.ascontiguousarray(inputs[f"w_{nm}"], np.float32)
            m[f"b_{nm}"] = np.ascontiguousarray(inputs[f"b_{nm}"], np.float32)
        in_maps.append(m)
    return in_maps


_NC = {}


def run(inputs, trace=False, trace_cores=None):
    if 8 not in _NC:
        _NC[8] = build_nc(8)
    res = run_bass_kernel_spmd(
        _NC[8], make_in_maps(inputs), core_ids=list(range(B)), trace=trace,
        trace_cores=trace_cores,
    )
    out = np.stack(
        [np.asarray(res.results[i]["adj"]).astype(np.float32) for i in range(B)],
        axis=0,
    )
    return out, res


def kernel(**inputs) -> np.ndarray:
    out, _ = run(inputs, trace=False)
    return out
